# revision 48
# baseline (speedup 1.0000x reference)
"""Trainium2 Bass kernel for ArccosHessianCalculator.

Math: for each batch element b (z1, z2 are [B, D] with D = 128):
  a = 1/|z1|, bb = 1/|z2|, c = cos = <z1u, z2u>
  Each Hessian block H_k is a rank-2 outer product plus a diagonal term:
      H_k(b) = z1 * r0_k(b)^T + z2 * r1_k(b)^T + diag-part
  where r0/r1 are per-element linear combinations of z1, z2 (all the
  normalization / cosine scale factors folded into the coefficients):
      k=0 (H11): r0 = -3c*a^4*z1 + a^3 b*z2          r1 = a^3 b*z1
      k=1 (H12): r0 = a^3 b*z1                        r1 = -c*a^2 b^2*z1 + a b^3*z2
      k=2 (H22): r0 = a b^3*z2                        r1 = a b^3*z1 - 3c*b^4*z2
  The (full, final) diagonals are computed separately in closed form and
  spliced in with a predicated copy against an identity mask.

Mapping to the chip (per core, batch shard of 512):
  - TensorE: one K=2 matmul per element, lhsT = [z1(b); z2(b)] ([2,128]),
    rhs = [r0 | r1] blocks ([2, 384]), streamed as float32r. In steady-state
    chunks, consecutive matmuls alternate PE row-quadrants (partition
    offsets 0/32 via the ZI/RI layout), so two matmuls execute concurrently
    on disjoint 32-row PE tiles.
  - ScalarE: PSUM -> SBUF staging copies, two elements per instruction
    (the element pair lands in the two banks of one 2-bank PSUM tile).
  - VectorE: stats + rhs coefficient builds + diagonal splice
    (copy_predicated, four consecutive elements per instruction).
  - DMA: batched 2MB output writes; steady-state chunks alternate between
    the sync HWDGE queue and the gpsimd SWDGE queue so descriptor
    generation (the co-limiter at ~1.15ns/desc on one HWDGE queue) runs on
    two queues in parallel.
The per-group stats work is software-pipelined: group g+1's stats are
emitted in four slices interleaved between group g's chunks.
Output per core: [3, 512, 128, 128] f32 (~100MB) -> DMA-bound overall.
"""

import numpy as np
from contextlib import ExitStack

import concourse.bass as bass
import concourse.tile as tile
from concourse.tile import add_dep_helper
from concourse import bacc, mybir
from concourse.bass_utils import run_bass_kernel_spmd

N_CORES = 8
B_FULL = 4096
D = 128
B_SH = B_FULL // N_CORES  # 512 batch elements per core
P = 128                   # SBUF partitions
KD = 3 * D                # 384: three H blocks side by side
F = 16                    # elements per partition-group row in ZI/RI tiles
G = 32                    # elements per gather chunk
GROUPS = B_SH // P        # 4 stats groups of 128 elements
NCH = P // G              # 4 chunks per group

f32 = mybir.dt.float32
f32r = mybir.dt.float32r
bf16 = mybir.dt.bfloat16
i32 = mybir.dt.int32


class _Pools:
    pass


def _make_pools(ctx, tc):
    p = _Pools()
    p.const = ctx.enter_context(tc.tile_pool(name="const", bufs=1))
    # bufs=2 on the stats pools doubles as a scheduler leash: group g+2's
    # stats can't start until group g's tiles are fully consumed by chunks
    p.zg = ctx.enter_context(tc.tile_pool(name="zg", bufs=2))
    p.work = ctx.enter_context(tc.tile_pool(name="work", bufs=2))
    p.stat = ctx.enter_context(tc.tile_pool(name="stat", bufs=3))
    p.rpool = ctx.enter_context(tc.tile_pool(name="rpool", bufs=2))
    p.dpool = ctx.enter_context(tc.tile_pool(name="dpool", bufs=2))
    p.zi = ctx.enter_context(tc.tile_pool(name="zi", bufs=2))
    p.ri = ctx.enter_context(tc.tile_pool(name="ri", bufs=3))
    p.stage = ctx.enter_context(tc.tile_pool(name="stage", bufs=3))
    # 2-bank PSUM tiles: each holds an element pair (cols 0:384 / 512:896).
    # The phase-3 transposes share this pool (tag "big") so they slot into
    # the matmul ring instead of hoisting ahead of chunk work.
    p.mmp = ctx.enter_context(tc.tile_pool(name="mmp", bufs=4, space="PSUM"))
    return p


def _emit_consts(p, nc):
    A = mybir.AluOpType
    colidx_i = p.const.tile([P, D], i32)
    nc.gpsimd.iota(colidx_i[:], [[1, D]], base=0, channel_multiplier=0)
    rowidx_i = p.const.tile([P, 1], i32)
    nc.gpsimd.iota(rowidx_i[:], [[0, 1]], base=0, channel_multiplier=1)
    colidx = p.const.tile([P, D], f32)
    nc.vector.tensor_copy(colidx[:], colidx_i[:])
    rowidx = p.const.tile([P, 1], f32)
    nc.vector.tensor_copy(rowidx[:], rowidx_i[:])
    eye = p.const.tile([P, D], f32)
    nc.vector.tensor_scalar(eye[:], colidx[:], rowidx[:], None, A.is_equal)
    # integer mask for copy_predicated (hw requires an int mask dtype);
    # eye12 = the [128,128] identity tiled 12x: covers a 4-element splice
    # (4 elements x 3 blocks); serial path slices the first 3 blocks.
    eyem = p.const.tile([P, D], mybir.dt.uint8)
    nc.vector.tensor_scalar(eyem[:], colidx[:], rowidx[:], None, A.is_equal)
    eye12 = p.const.tile([P, 4 * KD], mybir.dt.uint8)
    for r in range(12):
        nc.scalar.copy(eye12[:, r * D:(r + 1) * D], eyem[:])
    p.eye, p.eye12 = eye, eye12


def _pin(after, inst):
    """Scheduling-only edge: keep `inst` from being hoisted before `after`."""
    if after is not None:
        add_dep_helper(after.ins, inst.ins, sync=False,
                       reason="stats pinned behind chunk drain")


def _stats_phase0(p, nc, z1, z2, grp, use_f32r, after=None):
    """Loads, norms/cosine and the per-element scalar coefficient chain."""
    A = mybir.AluOpType
    st = {}
    b0 = grp * P
    # later groups' input loads go via gpsimd so they can't head-of-line
    # block group 0's gathers/output writes on the sync ring at startup
    ldma = nc.sync if grp == 0 else nc.gpsimd
    z1g = p.zg.tile([P, D], f32, tag="z1g", name=f"z1g_{grp}")
    _pin(after, ldma.dma_start(z1g[:], z1[b0:b0 + P, :]))
    z2g = p.zg.tile([P, D], f32, tag="z2g", name=f"z2g_{grp}")
    ldma.dma_start(z2g[:], z2[b0:b0 + P, :])

    def wt(tag):
        return p.work.tile([P, D], f32, tag=tag, name=f"w_{tag}_{grp}")

    def sv(tag):
        return p.stat.tile([P, 1], f32, tag=tag, name=f"sv_{tag}_{grp}")

    v1z, v2z, wz = wt("v1z"), wt("v2z"), wt("wz")
    s1, s2, dot = sv("s1"), sv("s2"), sv("dot")
    nc.vector.tensor_mul(v1z[:], z1g[:], z1g[:])
    nc.vector.tensor_mul(v2z[:], z2g[:], z2g[:])
    nc.vector.tensor_mul(wz[:], z1g[:], z2g[:])
    nc.vector.reduce_sum(s1[:], v1z[:], axis=mybir.AxisListType.X)
    nc.vector.reduce_sum(s2[:], v2z[:], axis=mybir.AxisListType.X)
    nc.vector.reduce_sum(dot[:], wz[:], axis=mybir.AxisListType.X)
    n1, n2 = sv("n1"), sv("n2")
    nc.scalar.sqrt(n1[:], s1[:])
    nc.scalar.sqrt(n2[:], s2[:])
    a, bb = sv("a"), sv("bb")
    nc.vector.reciprocal(a[:], n1[:])
    nc.vector.reciprocal(bb[:], n2[:])
    a2, b2, ab, c = sv("a2"), sv("b2"), sv("ab"), sv("c")
    nc.vector.tensor_mul(a2[:], a[:], a[:])
    nc.vector.tensor_mul(b2[:], bb[:], bb[:])
    nc.vector.tensor_mul(ab[:], a[:], bb[:])
    nc.vector.tensor_mul(c[:], dot[:], ab[:])
    m3c, mc = sv("m3c"), sv("mc")
    nc.vector.tensor_scalar(m3c[:], c[:], -3.0, None, A.mult)
    nc.vector.tensor_scalar(mc[:], c[:], -1.0, None, A.mult)
    A3B, AB3, A4, B4, A2B2 = sv("A3B"), sv("AB3"), sv("A4"), sv("B4"), sv("A2B2")
    nc.vector.tensor_mul(A3B[:], a2[:], ab[:])
    nc.vector.tensor_mul(AB3[:], b2[:], ab[:])
    nc.vector.tensor_mul(A4[:], a2[:], a2[:])
    nc.vector.tensor_mul(B4[:], b2[:], b2[:])
    nc.vector.tensor_mul(A2B2[:], ab[:], ab[:])
    m3cA4, m3cB4, mcA2B2, mcab = sv("m3cA4"), sv("m3cB4"), sv("mcA2B2"), sv("mcab")
    nc.vector.tensor_mul(m3cA4[:], A4[:], m3c[:])
    nc.vector.tensor_mul(m3cB4[:], B4[:], m3c[:])
    nc.vector.tensor_mul(mcA2B2[:], A2B2[:], mc[:])
    nc.vector.tensor_mul(mcab[:], ab[:], mc[:])
    # [128,1] scale factors for the ScalarE side of phase 2
    ab2, m3ca2, m3cb2 = sv("ab2"), sv("m3ca2"), sv("m3cb2")
    nc.vector.tensor_scalar(ab2[:], ab[:], 2.0, None, A.mult)
    nc.vector.tensor_mul(m3ca2[:], a2[:], m3c[:])
    nc.vector.tensor_mul(m3cb2[:], b2[:], m3c[:])

    # rounded copies of z1/z2 for the matmul lhsT gathers
    mmdt = f32r if use_f32r else f32
    z1r = p.zg.tile([P, D], mmdt, tag="z1r", name=f"z1r_{grp}")
    nc.vector.tensor_copy(z1r[:], z1g[:])
    z2r = p.zg.tile([P, D], mmdt, tag="z2r", name=f"z2r_{grp}")
    nc.vector.tensor_copy(z2r[:], z2g[:])

    st.update(z1g=z1g, z2g=z2g, v1z=v1z, v2z=v2z, wz=wz, a2=a2, b2=b2, ab=ab,
              c=c, m3c=m3c, A3B=A3B, AB3=AB3, m3cA4=m3cA4, m3cB4=m3cB4,
              mcA2B2=mcA2B2, mcab=mcab, z1r=z1r, z2r=z2r, wt=wt,
              ab2=ab2, m3ca2=m3ca2, m3cb2=m3cb2)
    return st


def _stats_phase1(p, nc, st, grp, use_f32r, after=None):
    """rhs rows R0, R1 [128b, 384] in float32r (rounded on DVE write)."""
    A = mybir.AluOpType
    mmdt = f32r if use_f32r else f32
    z1g, z2g, wt = st["z1g"], st["z2g"], st["wt"]
    A3B, AB3 = st["A3B"], st["AB3"]
    R0 = p.rpool.tile([P, KD], mmdt, tag="R0", name=f"R0_{grp}")
    R1 = p.rpool.tile([P, KD], mmdt, tag="R1", name=f"R1_{grp}")
    t0 = wt("t0")
    # plain per-partition-scaled mults ride ScalarE; DVE keeps the fused ops
    # k=0 (H11): r0 = m3cA4*z1 + A3B*z2 ; r1 = A3B*z1
    _pin(after, nc.scalar.mul(t0[:], z2g[:], A3B[:]))
    nc.vector.scalar_tensor_tensor(
        R0[:, 0:D], z1g[:], st["m3cA4"][:], t0[:], A.mult, A.add)
    nc.scalar.mul(R1[:, 0:D], z1g[:], A3B[:])
    # k=1 (H12): r0 = A3B*z1 ; r1 = mcA2B2*z1 + AB3*z2
    nc.scalar.mul(R0[:, D:2 * D], z1g[:], A3B[:])
    t1 = wt("t1")
    nc.scalar.mul(t1[:], z2g[:], AB3[:])
    nc.vector.scalar_tensor_tensor(
        R1[:, D:2 * D], z1g[:], st["mcA2B2"][:], t1[:], A.mult, A.add)
    # k=2 (H22): r0 = AB3*z2 ; r1 = AB3*z1 + m3cB4*z2
    nc.scalar.mul(R0[:, 2 * D:3 * D], z2g[:], AB3[:])
    t2 = wt("t2")
    nc.scalar.mul(t2[:], z2g[:], st["m3cB4"][:])
    nc.vector.scalar_tensor_tensor(
        R1[:, 2 * D:3 * D], z1g[:], AB3[:], t2[:], A.mult, A.add)
    st.update(R0=R0, R1=R1)


def _stats_phase2(p, nc, st, grp, after=None):
    """Final diagonal values, batch-major [128b, 128i]. Work is spread
    over ScalarE (per-partition-scaled copies), gpsimd (tensor+tensor
    adds) and DVE (fused ops) so no single engine eats the burst."""
    A = mybir.AluOpType
    wt = st["wt"]
    v1z, v2z, wz = st["v1z"], st["v2z"], st["wz"]
    a2, b2, ab, c = st["a2"], st["b2"], st["ab"], st["c"]
    twoabw = wt("twoabw")
    _pin(after, nc.scalar.mul(twoabw[:], wz[:], st["ab2"][:]))
    # d11 = a2*(c + 2ab*wz + m3c*a2*v1z)
    u1, u2 = wt("u1"), wt("u2")
    nc.scalar.mul(u1[:], v1z[:], st["m3ca2"][:])
    nc.vector.tensor_add(u2[:], u1[:], twoabw[:])
    d11 = p.dpool.tile([P, D], f32, tag="d11", name=f"d11_{grp}")
    nc.vector.tensor_scalar(d11[:], u2[:], c[:], a2[:], A.add, A.mult)
    # d22 = b2*(c + 2ab*wz + m3c*b2*v2z)
    u3, u4 = wt("u3"), wt("u4")
    nc.scalar.mul(u3[:], v2z[:], st["m3cb2"][:])
    nc.vector.tensor_add(u4[:], u3[:], twoabw[:])
    d22 = p.dpool.tile([P, D], f32, tag="d22", name=f"d22_{grp}")
    nc.vector.tensor_scalar(d22[:], u4[:], c[:], b2[:], A.add, A.mult)
    # d12 = ab*(a2*v1z + b2*v2z + mcab*wz - 1)
    w1, w2, w3 = wt("w1"), wt("w2"), wt("w3")
    nc.scalar.mul(w1[:], v1z[:], a2[:])
    nc.vector.scalar_tensor_tensor(w2[:], v2z[:], b2[:], w1[:], A.mult, A.add)
    nc.vector.scalar_tensor_tensor(w3[:], wz[:], st["mcab"][:], w2[:],
                                   A.mult, A.add)
    d12 = p.dpool.tile([P, D], f32, tag="d12", name=f"d12_{grp}")
    nc.vector.tensor_scalar(d12[:], w3[:], -1.0, ab[:], A.add, A.mult)
    st.update(d11=d11, d12=d12, d22=d22)


def _stats_phase3(p, nc, st, grp, after=None):
    """Transpose diagonals into [128i, 3*128b]."""
    diagT = p.dpool.tile([P, KD], bf16, tag="diagT", name=f"diagT_{grp}")
    for k, dk in enumerate([st["d11"], st["d12"], st["d22"]]):
        pt = p.mmp.tile([P, 1024], f32, tag="big", name=f"tpbig_{grp}_{k}")
        _pin(after, nc.tensor.transpose(pt[:, 0:D], dk[:], p.eye[:]))
        nc.scalar.copy(diagT[:, k * D:(k + 1) * D], pt[:, 0:D])
    st.update(diagT=diagT)


def _emit_gathers(p, nc, st, grp, ch, use_f32r):
    """Gather F batch rows into one partition row per (group, operand)."""
    mmdt = f32r if use_f32r else f32
    q0 = ch * G
    ZI = p.zi.tile([P, F * D], mmdt, tag="ZI", name=f"ZI_{grp}_{ch}")
    RI = p.ri.tile([P, F * KD], mmdt, tag="RI", name=f"RI_{grp}_{ch}")
    # gathers ride the (otherwise idle) gpsimd SWDGE path so the big output
    # writes on the sync HWDGE ring can't head-of-line block the next
    # chunk's operands; the very first chunk uses the still empty sync ring
    dmae = nc.sync if (grp == 0 and ch == 0) else nc.gpsimd
    z1r, z2r, R0, R1 = st["z1r"], st["z2r"], st["R0"], st["R1"]
    for g in range(2):
        qs = q0 + g * F
        dmae.dma_start(ZI[32 * g:32 * g + 1, :], z1r[qs:qs + F, :])
        dmae.dma_start(ZI[32 * g + 1:32 * g + 2, :], z2r[qs:qs + F, :])
        dmae.dma_start(RI[32 * g:32 * g + 1, :], R0[qs:qs + F, :])
        dmae.dma_start(RI[32 * g + 1:32 * g + 2, :], R1[qs:qs + F, :])
    return ZI, RI


def _emit_chunk_fast(p, nc, st, out, grp, ch, use_f32r):
    """Steady-state path: quadrant-interleaved matmuls (2 concurrent on
    disjoint PE row-tiles), 2-element ScalarE copies from 2-bank PSUM
    tiles, 4-element batched diagonal splices, half-chunk output drains."""
    b0 = grp * P
    e0 = b0 + ch * G
    q0 = ch * G
    ZI, RI = _emit_gathers(p, nc, st, grp, ch, use_f32r)

    diagT = st["diagT"]
    diagv = diagT[:].rearrange("p (k b one) -> p b k one", k=3, one=1)
    # element-major staging: contiguous per element for ScalarE/VectorE;
    # the [i, b, k, j] DRAM layout keeps output descriptors fat anyway
    STG = p.stage.tile([P, G * KD], bf16, tag="STG", name=f"STG_{grp}_{ch}_0")
    # [p, half, slot, n]: halves are the two PE quadrants (t, t+16)
    stgh = STG[:].rearrange("p (h e n) -> p h e n", h=2, n=KD)
    first_dma = None
    for t in range(F):
        big = p.mmp.tile([P, 1024], f32, tag="big", name=f"big_{grp}_{ch}_{t}")
        for h in range(2):
            pp = 32 * h
            lhsT = ZI[pp:pp + 2, t * D:(t + 1) * D]
            rhs = RI[pp:pp + 2, t * KD:(t + 1) * KD]
            nc.tensor.matmul(big[:, 512 * h:512 * h + KD], lhsT, rhs,
                             start=True, stop=True)
        src = big[:].rearrange("p (h j) -> p h j", h=2)[:, :, 0:KD]
        nc.scalar.copy(stgh[:, :, t, :], src)
        if t % 4 == 3:
            # splice the four just-copied slots of each quadrant, then
            # drain them (splice is the last STG writer, so the DMA's
            # semaphore wait resolves immediately after)
            for h in range(2):
                s0 = (t - 3) + F * h
                q = q0 + s0
                dst = STG[:, s0 * KD:(s0 + 4) * KD].rearrange(
                    "p (e k j) -> p e k j", e=4, k=3, j=D)
                datav = diagv[:, q:q + 4, :, :].broadcast_to([P, 4, 3, D])
                nc.vector.copy_predicated(dst, p.eye12[:], datav)
            for h in range(2):
                s0 = (t - 3) + F * h
                bs = e0 + s0
                dma = nc.sync.dma_start(out[:, bs:bs + 4, :, :],
                                        STG[:, s0 * KD:(s0 + 4) * KD])
                if first_dma is None:
                    first_dma = dma
    return first_dma


def _build_body(ctx, tc, z1, z2, out, use_f32r=True):
    nc = tc.nc
    p = _make_pools(ctx, tc)
    _emit_consts(p, nc)

    # group 0's stats run up front; each later group's stats are emitted in
    # slices between the previous group's chunks so the DVE/ACT burst is
    # amortized and no single chunk's compute exceeds its DMA drain window
    cur = _stats_phase0(p, nc, z1, z2, 0, use_f32r)
    _stats_phase1(p, nc, cur, 0, use_f32r)
    _stats_phase2(p, nc, cur, 0)
    _stats_phase3(p, nc, cur, 0)
    for grp in range(GROUPS):
        nxt = None
        for ch in range(NCH):
            dma0 = _emit_chunk_fast(p, nc, cur, out, grp, ch, use_f32r)
            if grp + 1 < GROUPS:
                # pin each stats phase behind the drain of the chunk it is
                # emitted after, so the scheduler can't front-load stats
                # ahead of chunk work (which would stall the first drains)
                if ch == 0:
                    nxt = _stats_phase0(p, nc, z1, z2, grp + 1, use_f32r,
                                        after=dma0)
                elif ch == 1:
                    _stats_phase1(p, nc, nxt, grp + 1, use_f32r, after=dma0)
                elif ch == 2:
                    _stats_phase2(p, nc, nxt, grp + 1, after=dma0)
                else:
                    _stats_phase3(p, nc, nxt, grp + 1, after=dma0)
        if nxt is not None:
            cur = nxt


def build_kernel(use_f32r=True):
    nc = bacc.Bacc("TRN2", target_bir_lowering=False, debug=False)
    z1 = nc.dram_tensor("z1", [B_SH, D], f32, kind="ExternalInput").ap()
    z2 = nc.dram_tensor("z2", [B_SH, D], f32, kind="ExternalInput").ap()
    # device layout is [i, b, k, j]: one output descriptor per partition
    # covers a contiguous (b, k, j) run; the host transposes back
    out = nc.dram_tensor("out", [D, B_SH, 3, D], bf16, kind="ExternalOutput").ap()
    with tile.TileContext(nc) as tc:
        with ExitStack() as ctx:
            _build_body(ctx, tc, z1, z2, out, use_f32r=use_f32r)
    nc.compile()
    return nc


_NC_CACHE = None


def _get_nc():
    global _NC_CACHE
    if _NC_CACHE is None:
        _NC_CACHE = build_kernel()
    return _NC_CACHE


def kernel(z1, z2):
    nc = _get_nc()
    z1 = np.ascontiguousarray(np.asarray(z1, dtype=np.float32))
    z2 = np.ascontiguousarray(np.asarray(z2, dtype=np.float32))
    in_maps = [
        {"z1": z1[c * B_SH:(c + 1) * B_SH], "z2": z2[c * B_SH:(c + 1) * B_SH]}
        for c in range(N_CORES)
    ]
    res = run_bass_kernel_spmd(nc, in_maps, core_ids=list(range(N_CORES)))
    # device buffers are bf16 [i, b, k, j]; reassemble to f32 [3, B, i, j]
    return np.concatenate(
        [np.asarray(res.results[c]["out"]).astype(np.float32).transpose(2, 1, 0, 3)
         for c in range(N_CORES)],
        axis=1)
